# revision 9
# baseline (speedup 1.0000x reference)
"""GraphSage layer-2 kernel for 8 TRN2 NeuronCores.

In the reference, h1/agg1/W1/... are dead code - the output is
softmax(LN2(relu([agg2 | hidden1[node_batch]] @ W2 + b2)) @ Wc' + bc') where
agg2 = masked-mean over feat rows gathered by neigh_idx[1], and g2/be2 fold
into Wc/bc host-side (Wc' = g2[:,None]*Wc, bc' = be2 @ Wc + bc).

Sharding: data-parallel over the 8192-row batch (1024 rows/core); feat and
hidden1 tables replicated in each core's HBM. Invalid neighbor slots are
redirected to an appended all-zero feat row so the weighted sum over the 10
slot gathers needs no masking of the gathered data itself.

This walrus build allows only ONE semaphore wait per instruction, so the
kernel is structured so every instruction depends on at most one
not-yet-observed producer: all float constants ride ONE packed DMA, metadata
rides one int32 DMA, warm-up ops absorb the constant waits per engine once,
the per-slot weighted aggregation is a serial DVE chain (each op waits only
its own gather), and PSUM->SBUF copies are split between ACT and DVE so PE
only ever waits one engine at a time.
"""

import numpy as np

import concourse.tile as tile
from concourse import bacc, bass, mybir
from concourse.bass_utils import run_bass_kernel_spmd
from concourse.masks import make_identity

N, F, B, S, H, C = 200000, 256, 8192, 10, 256, 64
NCORES = 8
BL = B // NCORES          # 1024 rows per core
P = 128
NT = BL // P              # 8 partition-tiles per core
NPAD = N + 8              # feat table padded; row N is all-zero
DIN = F + H               # 512
LN_EPS = 1e-5
F32 = mybir.dt.float32
I32 = mybir.dt.int32

# meta columns: [idx (S*NT, col=s*NT+t) | nbi (NT) | mask bits (NT*S, col=t*S+s)]
MC_IDX = 0
MC_NBI = S * NT
MC_MSK = S * NT + NT
MCOLS = S * NT + NT + NT * S

# wpack columns (f32): W2 chunks | Wc chunks | b2 bcast | bc bcast
WP_W2 = 0
WP_WC = 4 * H                 # 1024
WP_B2 = WP_WC + 2 * C         # 1152
WP_BC = WP_B2 + H             # 1408
WPCOLS = WP_BC + C            # 1472

_CACHE = {}


def _build():
    nc = bacc.Bacc()
    feat_d = nc.dram_tensor("feat", [NPAD, F], F32, kind="ExternalInput")
    hid_d = nc.dram_tensor("hid", [N, H], F32, kind="ExternalInput")
    meta_d = nc.dram_tensor("meta", [P, MCOLS], I32, kind="ExternalInput")
    wpack_d = nc.dram_tensor("wpack", [P, WPCOLS], F32, kind="ExternalInput")
    out_d = nc.dram_tensor("out", [BL, C], F32, kind="ExternalOutput")

    with tile.TileContext(nc) as tc:
        with tc.tile_pool(name="const", bufs=1) as const, \
             tc.tile_pool(name="gat", bufs=2) as gat, \
             tc.tile_pool(name="work", bufs=2) as work, \
             tc.tile_pool(name="small", bufs=4) as small, \
             tc.tile_pool(name="tps", bufs=6, space="PSUM") as tps, \
             tc.tile_pool(name="accp", bufs=1, space="PSUM") as accp, \
             tc.tile_pool(name="outp", bufs=1, space="PSUM") as outp:

            ident = const.tile([P, P], F32)
            make_identity(nc, ident[:])
            eps_t = const.tile([P, 1], F32)
            nc.vector.memset(eps_t[:], LN_EPS)
            wpack = const.tile([P, WPCOLS], F32)
            nc.sync.dma_start(out=wpack[:], in_=wpack_d[:])
            meta_sb = const.tile([P, MCOLS], I32)
            nc.sync.dma_start(out=meta_sb[:], in_=meta_d[:])

            def w2c(j):
                return wpack[:, WP_W2 + j * H: WP_W2 + (j + 1) * H]

            def wcc(j):
                return wpack[:, WP_WC + j * C: WP_WC + (j + 1) * C]

            b2_f = wpack[:, WP_B2:WP_B2 + H]
            bc_f = wpack[:, WP_BC:WP_BC + C]

            # ---- warm-ups: absorb constant-producer waits once per engine ----
            warm_t = tps.tile([P, P], F32, tag="tps")
            nc.tensor.transpose(out=warm_t[:], in_=ident[:], identity=ident[:])
            warm_o = outp.tile([P, C], F32, tag="o_ps")
            nc.tensor.matmul(out=warm_o[:C, :], lhsT=wcc(0)[:, :C],
                             rhs=wcc(1), start=True, stop=True)
            wmeta = small.tile([1, 1], I32, tag="wmeta")
            nc.vector.tensor_copy(out=wmeta[:], in_=meta_sb[0:1, 0:1])
            wwp = small.tile([1, 1], F32, tag="wwp")
            nc.vector.tensor_copy(out=wwp[:], in_=wpack[0:1, 0:1])
            for q in range(8):
                gw = gat.tile([P, F], F32, tag="gw", bufs=8, name=f"gw{q}")
                nc.gpsimd.indirect_dma_start(
                    out=gw[:], out_offset=None, in_=feat_d[:],
                    in_offset=bass.IndirectOffsetOnAxis(
                        ap=meta_sb[:, q:q + 1], axis=0))

            for t in range(NT):
                # ---- gathers: 10 neighbor-slot tiles + self rows ----
                gs = []
                for s in range(S):
                    g = gat.tile([P, F], F32, tag=f"g{s}", name=f"g{s}")
                    col = MC_IDX + s * NT + t
                    nc.gpsimd.indirect_dma_start(
                        out=g[:], out_offset=None, in_=feat_d[:],
                        in_offset=bass.IndirectOffsetOnAxis(
                            ap=meta_sb[:, col:col + 1], axis=0))
                    gs.append(g)
                selfh = gat.tile([P, H], F32, tag="selfh")
                nc.gpsimd.indirect_dma_start(
                    out=selfh[:], out_offset=None, in_=hid_d[:],
                    in_offset=bass.IndirectOffsetOnAxis(
                        ap=meta_sb[:, MC_NBI + t:MC_NBI + t + 1], axis=0))

                # ---- neighbor weights: wv = mask / max(cnt,1)  [DVE] ----
                mrow = meta_sb[:, MC_MSK + t * S: MC_MSK + (t + 1) * S] \
                    .bitcast(F32)
                cnt = small.tile([P, 1], F32, tag="cnt")
                nc.vector.tensor_reduce(
                    out=cnt[:], in_=mrow,
                    axis=mybir.AxisListType.X, op=mybir.AluOpType.add)
                nc.vector.tensor_scalar_max(out=cnt[:], in0=cnt[:], scalar1=1.0)
                inv = small.tile([P, 1], F32, tag="inv")
                nc.vector.reciprocal(out=inv[:], in_=cnt[:])
                wv = small.tile([P, S], F32, tag="wv")
                nc.vector.tensor_scalar_mul(out=wv[:], in0=mrow, scalar1=inv[:])

                # ---- weighted aggregation chain [DVE]: acc = sum_s wv_s*G_s ----
                acc = work.tile([P, F], F32, tag="acc")
                nc.vector.tensor_scalar_mul(
                    out=acc[:], in0=gs[0][:], scalar1=wv[:, 0:1])
                for s in range(1, S):
                    nc.vector.scalar_tensor_tensor(
                        out=acc[:], in0=gs[s][:], scalar=wv[:, s:s + 1],
                        in1=acc[:], op0=mybir.AluOpType.mult,
                        op1=mybir.AluOpType.add)
                # self rows chained through ACT so PE never waits a DMA queue
                selfc = work.tile([P, H], F32, tag="selfc")
                nc.scalar.copy(out=selfc[:], in_=selfh[:])

                # ---- x2T chunks via PE transposes; copies split ACT/DVE ----
                x2t = work.tile([P, 4, P], F32, tag="x2t")
                for j in range(4):
                    src = acc if j < 2 else selfc
                    jj = j % 2
                    tp = tps.tile([P, P], F32, tag="tps", name=f"tp{j}")
                    nc.tensor.transpose(
                        out=tp[:], in_=src[:, jj * P:(jj + 1) * P],
                        identity=ident[:])
                    eng = nc.scalar if j % 2 == 0 else nc.vector
                    if eng is nc.scalar:
                        nc.scalar.copy(out=x2t[:, j, :], in_=tp[:])
                    else:
                        nc.vector.tensor_copy(out=x2t[:, j, :], in_=tp[:])

                # ---- h = relu(x2 @ W2 + b2) ----
                h_ps = accp.tile([P, H], F32, tag="h_ps")
                for j in range(4):
                    nc.tensor.matmul(
                        out=h_ps[:], lhsT=x2t[:, j, :], rhs=w2c(j),
                        start=(j == 0), stop=(j == 3))
                h_sb = work.tile([P, H], F32, tag="h_sb")
                nc.vector.tensor_tensor(
                    out=h_sb[:], in0=h_ps[:], in1=b2_f,
                    op=mybir.AluOpType.add)
                nc.scalar.activation(
                    out=h_sb[:], in_=h_sb[:],
                    func=mybir.ActivationFunctionType.Relu)

                # ---- layernorm (g/be folded into Wc'/bc' on host) [DVE] ----
                stats = small.tile([P, 6], F32, tag="stats")
                nc.vector.bn_stats(out=stats[:], in_=h_sb[:])
                mv = small.tile([P, 2], F32, tag="mv")
                nc.vector.bn_aggr(out=mv[:], in_=stats[:])
                std = small.tile([P, 1], F32, tag="std")
                nc.scalar.activation(
                    out=std[:], in_=mv[:, 1:2],
                    func=mybir.ActivationFunctionType.Sqrt, bias=eps_t[:])
                rstd = small.tile([P, 1], F32, tag="rstd")
                nc.vector.reciprocal(out=rstd[:], in_=std[:])
                xn = work.tile([P, H], F32, tag="xn")
                nc.vector.tensor_scalar(
                    out=xn[:], in0=h_sb[:],
                    scalar1=mv[:, 0:1], scalar2=rstd[:],
                    op0=mybir.AluOpType.subtract, op1=mybir.AluOpType.mult)

                # ---- logits = xn @ Wc' + bc' ----
                h2t = work.tile([P, 2, P], F32, tag="h2t")
                for j in range(2):
                    tp2 = tps.tile([P, P], F32, tag="tps", name=f"tp2{j}")
                    nc.tensor.transpose(
                        out=tp2[:], in_=xn[:, j * P:(j + 1) * P],
                        identity=ident[:])
                    if j == 0:
                        nc.scalar.copy(out=h2t[:, j, :], in_=tp2[:])
                    else:
                        nc.vector.tensor_copy(out=h2t[:, j, :], in_=tp2[:])
                o_ps = outp.tile([P, C], F32, tag="o_ps")
                for j in range(2):
                    nc.tensor.matmul(
                        out=o_ps[:], lhsT=h2t[:, j, :], rhs=wcc(j),
                        start=(j == 0), stop=(j == 1))
                ob = work.tile([P, C], F32, tag="ob")
                nc.vector.tensor_tensor(
                    out=ob[:], in0=o_ps[:], in1=bc_f,
                    op=mybir.AluOpType.add)

                # ---- softmax ----
                mx = small.tile([P, 1], F32, tag="mx")
                nc.vector.tensor_reduce(
                    out=mx[:], in_=ob[:],
                    axis=mybir.AxisListType.X, op=mybir.AluOpType.max)
                nmx = small.tile([P, 1], F32, tag="nmx")
                nc.vector.tensor_scalar_mul(out=nmx[:], in0=mx[:], scalar1=-1.0)
                esb = work.tile([P, C], F32, tag="esb")
                ssum = small.tile([P, 1], F32, tag="ssum")
                nc.scalar.activation(
                    out=esb[:], in_=ob[:],
                    func=mybir.ActivationFunctionType.Exp,
                    bias=nmx[:], accum_out=ssum[:])
                rsum = small.tile([P, 1], F32, tag="rsum")
                nc.vector.reciprocal(out=rsum[:], in_=ssum[:])
                res = work.tile([P, C], F32, tag="res")
                nc.vector.tensor_scalar_mul(out=res[:], in0=esb[:], scalar1=rsum[:])
                nc.sync.dma_start(out=out_d[t * P:(t + 1) * P, :], in_=res[:])

    nc.compile()
    return nc


def _get_nc():
    if "nc" not in _CACHE:
        _CACHE["nc"] = _build()
    return _CACHE["nc"]


def _prep_inputs(node_batch, neigh_idx, neigh_mask, feat, hidden1,
                 W2, b2, g2, be2, Wc, bc):
    node_batch = np.asarray(node_batch).astype(np.int32)
    idx2 = np.asarray(neigh_idx[1]).astype(np.int32)        # [B, S]
    m2 = np.asarray(neigh_mask[1]).astype(bool)             # [B, S]
    feat = np.asarray(feat, dtype=np.float32)
    hidden1 = np.asarray(hidden1, dtype=np.float32)
    W2 = np.asarray(W2, dtype=np.float32)
    b2 = np.asarray(b2, dtype=np.float32)
    g2 = np.asarray(g2, dtype=np.float32)
    be2 = np.asarray(be2, dtype=np.float32)
    Wc = np.asarray(Wc, dtype=np.float32)
    bc = np.asarray(bc, dtype=np.float32)

    feat_ext = np.zeros((NPAD, F), np.float32)
    feat_ext[:N] = feat
    idx_eff = np.where(m2, idx2, N).astype(np.int32)        # invalid -> zero row
    wc_p = (g2[:, None] * Wc).astype(np.float32)
    bc_p = (be2 @ Wc + bc).astype(np.float32)
    mask_f = m2.astype(np.float32)

    wpack = np.empty((P, WPCOLS), np.float32)
    wpack[:, WP_W2:WP_WC] = W2.reshape(4, P, H).transpose(1, 0, 2) \
                              .reshape(P, 4 * H)
    wpack[:, WP_WC:WP_B2] = wc_p.reshape(2, P, C).transpose(1, 0, 2) \
                                .reshape(P, 2 * C)
    wpack[:, WP_B2:WP_BC] = np.broadcast_to(b2, (P, H))
    wpack[:, WP_BC:] = np.broadcast_to(bc_p, (P, C))

    in_maps = []
    for c in range(NCORES):
        lo = c * BL
        meta = np.empty((P, MCOLS), np.int32)
        meta[:, MC_IDX:MC_NBI] = (
            idx_eff[lo:lo + BL].reshape(NT, P, S).transpose(1, 2, 0)
            .reshape(P, S * NT))
        meta[:, MC_NBI:MC_MSK] = node_batch[lo:lo + BL].reshape(NT, P).T
        meta[:, MC_MSK:] = (
            mask_f[lo:lo + BL].reshape(NT, P, S).transpose(1, 0, 2)
            .reshape(P, NT * S).view(np.int32))
        in_maps.append({
            "feat": feat_ext, "hid": hidden1, "meta": meta, "wpack": wpack,
        })
    return in_maps


def kernel(node_batch, neigh_idx, neigh_mask, feat, hidden1,
           W1, b1, g1, be1, W2, b2, g2, be2, Wc, bc, **extra):
    in_maps = _prep_inputs(node_batch, neigh_idx, neigh_mask, feat, hidden1,
                           W2, b2, g2, be2, Wc, bc)
    nc = _get_nc()
    r = run_bass_kernel_spmd(nc, in_maps, core_ids=list(range(NCORES)),
                             **_CACHE.get("run_kwargs", {}))
    out = np.concatenate([r.results[c]["out"] for c in range(NCORES)], axis=0)
    _CACHE["last_result"] = r
    return out


# revision 11
# speedup vs baseline: 1.1110x; 1.1110x over previous
"""GraphSage layer-2 kernel for 8 TRN2 NeuronCores.

In the reference, h1/agg1/W1/... are dead code - the output is
softmax(LN2(relu([agg2 | hidden1[node_batch]] @ W2 + b2)) @ Wc' + bc') where
agg2 = masked-mean over feat rows gathered by neigh_idx[1], and g2/be2 fold
into Wc/bc host-side (Wc' = g2[:,None]*Wc, bc' = be2 @ Wc + bc).

Sharding: data-parallel over the 8192-row batch (1024 rows/core); feat and
hidden1 tables replicated in each core's HBM. Invalid neighbor slots are
redirected to an appended all-zero feat row so the weighted sum over the 10
slot gathers needs no masking of the gathered data itself.

This walrus build allows only ONE semaphore wait per instruction, so the
kernel is structured so every instruction depends on at most one
not-yet-observed producer: all float constants ride ONE packed DMA, metadata
rides one int32 DMA, warm-up ops absorb the constant waits per engine once,
the per-slot weighted aggregation is a serial DVE chain (each op waits only
its own gather), and PSUM->SBUF copies are split between ACT and DVE so PE
only ever waits one engine at a time.
"""

import numpy as np

import concourse.tile as tile
from concourse import bacc, bass, mybir
from concourse.bass_utils import run_bass_kernel_spmd
from concourse.masks import make_identity

N, F, B, S, H, C = 200000, 256, 8192, 10, 256, 64
NCORES = 8
BL = B // NCORES          # 1024 rows per core
P = 128
NT = BL // P              # 8 partition-tiles per core
NPAD = N + 8              # feat table padded; row N is all-zero
DIN = F + H               # 512
LN_EPS = 1e-5
F32 = mybir.dt.float32
I32 = mybir.dt.int32

# meta columns: [idx (S*NT, col=s*NT+t) | nbi (NT) | mask bits (NT*S, col=t*S+s)]
MC_IDX = 0
MC_NBI = S * NT
MC_MSK = S * NT + NT
MCOLS = S * NT + NT + NT * S

# wpack columns (f32): W2 chunks | Wc chunks | b2 bcast | bc bcast
WP_W2 = 0
WP_WC = 4 * H                 # 1024
WP_B2 = WP_WC + 2 * C         # 1152
WP_BC = WP_B2 + H             # 1408
WPCOLS = WP_BC + C            # 1472

_CACHE = {}


def _build():
    nc = bacc.Bacc()
    feat_d = nc.dram_tensor("feat", [NPAD, F], F32, kind="ExternalInput")
    hid_d = nc.dram_tensor("hid", [N, H], F32, kind="ExternalInput")
    meta_d = nc.dram_tensor("meta", [P, MCOLS], I32, kind="ExternalInput")
    wpack_d = nc.dram_tensor("wpack", [P, WPCOLS], F32, kind="ExternalInput")
    out_d = nc.dram_tensor("out", [BL, C], F32, kind="ExternalOutput")

    with tile.TileContext(nc) as tc:
        with tc.tile_pool(name="const", bufs=1) as const, \
             tc.tile_pool(name="gat", bufs=2) as gat, \
             tc.tile_pool(name="work", bufs=2) as work, \
             tc.tile_pool(name="small", bufs=4) as small, \
             tc.tile_pool(name="tps", bufs=4, space="PSUM") as tps, \
             tc.tile_pool(name="accp", bufs=2, space="PSUM") as accp, \
             tc.tile_pool(name="outp", bufs=2, space="PSUM") as outp:

            ident = const.tile([P, P], F32)
            make_identity(nc, ident[:])
            eps_t = const.tile([P, 1], F32)
            nc.vector.memset(eps_t[:], LN_EPS)
            wpack = const.tile([P, WPCOLS], F32)
            nc.sync.dma_start(out=wpack[:], in_=wpack_d[:])
            meta_sb = const.tile([P, MCOLS], I32)
            nc.sync.dma_start(out=meta_sb[:], in_=meta_d[:])

            def w2c(j):
                return wpack[:, WP_W2 + j * H: WP_W2 + (j + 1) * H]

            def wcc(j):
                return wpack[:, WP_WC + j * C: WP_WC + (j + 1) * C]

            b2_f = wpack[:, WP_B2:WP_B2 + H]
            bc_f = wpack[:, WP_BC:WP_BC + C]


            for t in range(NT):
                # ---- gathers: 10 neighbor-slot tiles + self rows ----
                gs = []
                for s in range(S):
                    g = gat.tile([P, F], F32, tag=f"g{s}", name=f"g{s}", bufs=3)
                    col = MC_IDX + t * S + s
                    nc.gpsimd.indirect_dma_start(
                        out=g[:], out_offset=None, in_=feat_d[:],
                        in_offset=bass.IndirectOffsetOnAxis(
                            ap=meta_sb[:, col:col + 1], axis=0))
                    gs.append(g)
                selfh = gat.tile([P, H], F32, tag="selfh", bufs=3)
                nc.gpsimd.indirect_dma_start(
                    out=selfh[:], out_offset=None, in_=hid_d[:],
                    in_offset=bass.IndirectOffsetOnAxis(
                        ap=meta_sb[:, MC_NBI + t:MC_NBI + t + 1], axis=0))

                # ---- neighbor weights: wv = mask / max(cnt,1)  [DVE] ----
                mrow = meta_sb[:, MC_MSK + t * S: MC_MSK + (t + 1) * S] \
                    .bitcast(F32)
                cnt = small.tile([P, 1], F32, tag="cnt")
                nc.vector.tensor_reduce(
                    out=cnt[:], in_=mrow,
                    axis=mybir.AxisListType.X, op=mybir.AluOpType.add)
                nc.vector.tensor_scalar_max(out=cnt[:], in0=cnt[:], scalar1=1.0)
                inv = small.tile([P, 1], F32, tag="inv")
                nc.vector.reciprocal(out=inv[:], in_=cnt[:])
                wv = small.tile([P, S], F32, tag="wv")
                nc.vector.tensor_scalar_mul(out=wv[:], in0=mrow, scalar1=inv[:])

                # ---- weighted aggregation chain [DVE]: acc = sum_s wv_s*G_s ----
                acc = work.tile([P, F], F32, tag="acc")
                nc.vector.tensor_scalar_mul(
                    out=acc[:], in0=gs[0][:], scalar1=wv[:, 0:1])
                for s in range(1, S):
                    nc.vector.scalar_tensor_tensor(
                        out=acc[:], in0=gs[s][:], scalar=wv[:, s:s + 1],
                        in1=acc[:], op0=mybir.AluOpType.mult,
                        op1=mybir.AluOpType.add)

                # ---- x2T chunks via PE transposes; copies split ACT/DVE ----
                x2t = work.tile([P, 4, P], F32, tag="x2t")
                for j in range(4):
                    jj = j % 2
                    srcap = (acc[:, jj * P:(jj + 1) * P] if j < 2
                             else selfh[:, jj * P:(jj + 1) * P])
                    tp = tps.tile([P, P], F32, tag="tps", name=f"tp{j}")
                    nc.tensor.transpose(
                        out=tp[:], in_=srcap, identity=ident[:])
                    eng = nc.scalar if j % 2 == 0 else nc.vector
                    if eng is nc.scalar:
                        nc.scalar.copy(out=x2t[:, j, :], in_=tp[:])
                    else:
                        nc.vector.tensor_copy(out=x2t[:, j, :], in_=tp[:])

                # ---- h = relu(x2 @ W2 + b2) ----
                h_ps = accp.tile([P, H], F32, tag="h_ps")
                for j in range(4):
                    nc.tensor.matmul(
                        out=h_ps[:], lhsT=x2t[:, j, :], rhs=w2c(j),
                        start=(j == 0), stop=(j == 3))
                h_sb = work.tile([P, H], F32, tag="h_sb")
                nc.vector.tensor_tensor(
                    out=h_sb[:], in0=h_ps[:], in1=b2_f,
                    op=mybir.AluOpType.add)
                nc.vector.tensor_scalar_max(
                    out=h_sb[:], in0=h_sb[:], scalar1=0.0)

                # ---- layernorm (g/be folded into Wc'/bc' on host) [DVE] ----
                stats = small.tile([P, 6], F32, tag="stats")
                nc.vector.bn_stats(out=stats[:], in_=h_sb[:])
                mv = small.tile([P, 2], F32, tag="mv")
                nc.vector.bn_aggr(out=mv[:], in_=stats[:])
                std = small.tile([P, 1], F32, tag="std")
                nc.scalar.activation(
                    out=std[:], in_=mv[:, 1:2],
                    func=mybir.ActivationFunctionType.Sqrt, bias=eps_t[:])
                rstd = small.tile([P, 1], F32, tag="rstd")
                nc.vector.reciprocal(out=rstd[:], in_=std[:])
                xn = work.tile([P, H], F32, tag="xn")
                nc.vector.tensor_scalar(
                    out=xn[:], in0=h_sb[:],
                    scalar1=mv[:, 0:1], scalar2=rstd[:],
                    op0=mybir.AluOpType.subtract, op1=mybir.AluOpType.mult)

                # ---- logits = xn @ Wc' + bc' ----
                h2t = work.tile([P, 2, P], F32, tag="h2t")
                for j in range(2):
                    tp2 = tps.tile([P, P], F32, tag="tps", name=f"tp2{j}")
                    nc.tensor.transpose(
                        out=tp2[:], in_=xn[:, j * P:(j + 1) * P],
                        identity=ident[:])
                    if j == 0:
                        nc.scalar.copy(out=h2t[:, j, :], in_=tp2[:])
                    else:
                        nc.vector.tensor_copy(out=h2t[:, j, :], in_=tp2[:])
                o_ps = outp.tile([P, C], F32, tag="o_ps")
                for j in range(2):
                    nc.tensor.matmul(
                        out=o_ps[:], lhsT=h2t[:, j, :], rhs=wcc(j),
                        start=(j == 0), stop=(j == 1))
                ob = work.tile([P, C], F32, tag="ob")
                nc.vector.tensor_tensor(
                    out=ob[:], in0=o_ps[:], in1=bc_f,
                    op=mybir.AluOpType.add)

                # ---- softmax ----
                mx = small.tile([P, 1], F32, tag="mx")
                nc.vector.tensor_reduce(
                    out=mx[:], in_=ob[:],
                    axis=mybir.AxisListType.X, op=mybir.AluOpType.max)
                nmx = small.tile([P, 1], F32, tag="nmx")
                nc.vector.tensor_scalar_mul(out=nmx[:], in0=mx[:], scalar1=-1.0)
                esb = work.tile([P, C], F32, tag="esb")
                ssum = small.tile([P, 1], F32, tag="ssum")
                nc.scalar.activation(
                    out=esb[:], in_=ob[:],
                    func=mybir.ActivationFunctionType.Exp,
                    bias=nmx[:], accum_out=ssum[:])
                rsum = small.tile([P, 1], F32, tag="rsum")
                nc.vector.reciprocal(out=rsum[:], in_=ssum[:])
                res = work.tile([P, C], F32, tag="res")
                nc.vector.tensor_scalar_mul(out=res[:], in0=esb[:], scalar1=rsum[:])
                nc.sync.dma_start(out=out_d[t * P:(t + 1) * P, :], in_=res[:])

    nc.compile()
    return nc


def _get_nc():
    if "nc" not in _CACHE:
        _CACHE["nc"] = _build()
    return _CACHE["nc"]


def _prep_inputs(node_batch, neigh_idx, neigh_mask, feat, hidden1,
                 W2, b2, g2, be2, Wc, bc):
    node_batch = np.asarray(node_batch).astype(np.int32)
    idx2 = np.asarray(neigh_idx[1]).astype(np.int32)        # [B, S]
    m2 = np.asarray(neigh_mask[1]).astype(bool)             # [B, S]
    feat = np.asarray(feat, dtype=np.float32)
    hidden1 = np.asarray(hidden1, dtype=np.float32)
    W2 = np.asarray(W2, dtype=np.float32)
    b2 = np.asarray(b2, dtype=np.float32)
    g2 = np.asarray(g2, dtype=np.float32)
    be2 = np.asarray(be2, dtype=np.float32)
    Wc = np.asarray(Wc, dtype=np.float32)
    bc = np.asarray(bc, dtype=np.float32)

    feat_ext = np.zeros((NPAD, F), np.float32)
    feat_ext[:N] = feat
    idx_eff = np.where(m2, idx2, N).astype(np.int32)        # invalid -> zero row
    wc_p = (g2[:, None] * Wc).astype(np.float32)
    bc_p = (be2 @ Wc + bc).astype(np.float32)
    mask_f = m2.astype(np.float32)

    wpack = np.empty((P, WPCOLS), np.float32)
    wpack[:, WP_W2:WP_WC] = W2.reshape(4, P, H).transpose(1, 0, 2) \
                              .reshape(P, 4 * H)
    wpack[:, WP_WC:WP_B2] = wc_p.reshape(2, P, C).transpose(1, 0, 2) \
                                .reshape(P, 2 * C)
    wpack[:, WP_B2:WP_BC] = np.broadcast_to(b2, (P, H))
    wpack[:, WP_BC:] = np.broadcast_to(bc_p, (P, C))

    in_maps = []
    for c in range(NCORES):
        lo = c * BL
        meta = np.empty((P, MCOLS), np.int32)
        meta[:, MC_IDX:MC_NBI] = (
            idx_eff[lo:lo + BL].reshape(NT, P, S).transpose(1, 0, 2)
            .reshape(P, NT * S))
        meta[:, MC_NBI:MC_MSK] = node_batch[lo:lo + BL].reshape(NT, P).T
        meta[:, MC_MSK:] = (
            mask_f[lo:lo + BL].reshape(NT, P, S).transpose(1, 0, 2)
            .reshape(P, NT * S).view(np.int32))
        in_maps.append({
            "feat": feat_ext, "hid": hidden1, "meta": meta, "wpack": wpack,
        })
    return in_maps


def kernel(node_batch, neigh_idx, neigh_mask, feat, hidden1,
           W1, b1, g1, be1, W2, b2, g2, be2, Wc, bc, **extra):
    in_maps = _prep_inputs(node_batch, neigh_idx, neigh_mask, feat, hidden1,
                           W2, b2, g2, be2, Wc, bc)
    nc = _get_nc()
    r = run_bass_kernel_spmd(nc, in_maps, core_ids=list(range(NCORES)),
                             **_CACHE.get("run_kwargs", {}))
    out = np.concatenate([r.results[c]["out"] for c in range(NCORES)], axis=0)
    _CACHE["last_result"] = r
    return out


# revision 14
# speedup vs baseline: 1.1587x; 1.0429x over previous
"""GraphSage layer-2 kernel for 8 TRN2 NeuronCores.

In the reference, h1/agg1/W1/... are dead code - the output is
softmax(LN2(relu([agg2 | hidden1[node_batch]] @ W2 + b2)) @ Wc' + bc') where
agg2 = masked-mean over feat rows gathered by neigh_idx[1], and g2/be2 fold
into Wc/bc host-side (Wc' = g2[:,None]*Wc, bc' = be2 @ Wc + bc).

Sharding: data-parallel over the 8192-row batch (1024 rows/core); feat and
hidden1 tables replicated in each core's HBM. Invalid neighbor slots are
redirected to an appended all-zero feat row so the weighted sum over the 10
slot gathers needs no masking of the gathered data itself.

This walrus build allows only ONE semaphore wait per instruction, so the
kernel is structured so every instruction depends on at most one
not-yet-observed producer: all float constants ride ONE packed DMA, metadata
rides one int32 DMA, warm-up ops absorb the constant waits per engine once,
the per-slot weighted aggregation is a serial DVE chain (each op waits only
its own gather), and PSUM->SBUF copies are split between ACT and DVE so PE
only ever waits one engine at a time.
"""

import numpy as np

import concourse.tile as tile
from concourse import bacc, bass, mybir
from concourse.bass_utils import run_bass_kernel_spmd
from concourse.masks import make_identity

N, F, B, S, H, C = 200000, 256, 8192, 10, 256, 64
NCORES = 8
BL = B // NCORES          # 1024 rows per core
P = 128
NT = BL // P              # 8 partition-tiles per core
NPAD = N + 8              # feat table padded; row N is all-zero
DIN = F + H               # 512
LN_EPS = 1e-5
F32 = mybir.dt.float32
BF16 = mybir.dt.bfloat16
I32 = mybir.dt.int32

# meta columns: [idx (S*NT, col=s*NT+t) | nbi (NT) | mask bits (NT*S, col=t*S+s)]
MC_IDX = 0
MC_NBI = S * NT
MC_MSK = S * NT + NT
MCOLS = S * NT + NT + NT * S

# wpack columns (f32): W2 chunks | Wc chunks | b2 bcast | bc bcast
WP_W2 = 0
WP_WC = 4 * H                 # 1024
WP_B2 = WP_WC + 2 * C         # 1152
WP_BC = WP_B2 + H             # 1408
WPCOLS = WP_BC + C            # 1472

_CACHE = {}


def _build():
    nc = bacc.Bacc()
    feat_d = nc.dram_tensor("feat", [NPAD, F], BF16, kind="ExternalInput")
    hid_d = nc.dram_tensor("hid", [N, H], BF16, kind="ExternalInput")
    meta_d = nc.dram_tensor("meta", [P, MCOLS], I32, kind="ExternalInput")
    wpack_d = nc.dram_tensor("wpack", [P, WPCOLS], BF16, kind="ExternalInput")
    bpack_d = nc.dram_tensor("bpack", [P, H + C], F32, kind="ExternalInput")
    out_d = nc.dram_tensor("out", [BL, C], F32, kind="ExternalOutput")

    with tile.TileContext(nc) as tc:
        with tc.tile_pool(name="const", bufs=1) as const, \
             tc.tile_pool(name="gat", bufs=2) as gat, \
             tc.tile_pool(name="work", bufs=2) as work, \
             tc.tile_pool(name="small", bufs=4) as small, \
             tc.tile_pool(name="tps", bufs=4, space="PSUM") as tps, \
             tc.tile_pool(name="accp", bufs=2, space="PSUM") as accp, \
             tc.tile_pool(name="outp", bufs=2, space="PSUM") as outp:

            ident = const.tile([P, P], F32)
            make_identity(nc, ident[:])
            ident_bf = const.tile([P, P], BF16)
            make_identity(nc, ident_bf[:])
            eps_t = const.tile([P, 1], F32)
            nc.vector.memset(eps_t[:], LN_EPS)
            wpack = const.tile([P, WPCOLS], BF16)
            nc.sync.dma_start(out=wpack[:], in_=wpack_d[:])
            bpack = const.tile([P, H + C], F32)
            nc.sync.dma_start(out=bpack[:], in_=bpack_d[:])
            meta_sb = const.tile([P, MCOLS], I32)
            nc.sync.dma_start(out=meta_sb[:], in_=meta_d[:])

            def w2c(j):
                return wpack[:, WP_W2 + j * H: WP_W2 + (j + 1) * H]

            def wcc(j):
                return wpack[:, WP_WC + j * C: WP_WC + (j + 1) * C]

            b2_f = bpack[:, 0:H]
            bc_f = bpack[:, H:H + C]


            for t in range(NT):
                # ---- gathers: 10 neighbor-slot tiles + self rows ----
                gs = []
                for s in range(S):
                    g = gat.tile([P, F], BF16, tag=f"g{s}", name=f"g{s}", bufs=3)
                    col = MC_IDX + t * S + s
                    nc.gpsimd.indirect_dma_start(
                        out=g[:], out_offset=None, in_=feat_d[:],
                        in_offset=bass.IndirectOffsetOnAxis(
                            ap=meta_sb[:, col:col + 1], axis=0))
                    gs.append(g)
                selfh = gat.tile([P, H], BF16, tag="selfh", bufs=3)
                nc.gpsimd.indirect_dma_start(
                    out=selfh[:], out_offset=None, in_=hid_d[:],
                    in_offset=bass.IndirectOffsetOnAxis(
                        ap=meta_sb[:, MC_NBI + t:MC_NBI + t + 1], axis=0))

                # ---- neighbor weights: wv = mask / max(cnt,1)  [DVE] ----
                mrow = meta_sb[:, MC_MSK + t * S: MC_MSK + (t + 1) * S] \
                    .bitcast(F32)
                cnt = small.tile([P, 1], F32, tag="cnt")
                nc.vector.tensor_reduce(
                    out=cnt[:], in_=mrow,
                    axis=mybir.AxisListType.X, op=mybir.AluOpType.add)
                nc.vector.tensor_scalar_max(out=cnt[:], in0=cnt[:], scalar1=1.0)
                inv = small.tile([P, 1], F32, tag="inv")
                nc.vector.reciprocal(out=inv[:], in_=cnt[:])
                wv = small.tile([P, S], F32, tag="wv")
                nc.vector.tensor_scalar_mul(out=wv[:], in0=mrow, scalar1=inv[:])

                # ---- weighted aggregation chain [DVE]: acc = sum_s wv_s*G_s ----
                acc = work.tile([P, F], F32, tag="acc")
                nc.vector.tensor_scalar_mul(
                    out=acc[:], in0=gs[0][:], scalar1=wv[:, 0:1])
                for s in range(1, S):
                    nc.vector.scalar_tensor_tensor(
                        out=acc[:], in0=gs[s][:], scalar=wv[:, s:s + 1],
                        in1=acc[:], op0=mybir.AluOpType.mult,
                        op1=mybir.AluOpType.add)

                # ---- x2T chunks via PE transposes; copies split ACT/DVE ----
                x2t = work.tile([P, 4, P], BF16, tag="x2t")
                for j in range(4):
                    jj = j % 2
                    srcap = (acc[:, jj * P:(jj + 1) * P] if j < 2
                             else selfh[:, jj * P:(jj + 1) * P])
                    tp = tps.tile([P, P], F32 if j < 2 else BF16,
                                  tag="tps", name=f"tp{j}")
                    nc.tensor.transpose(
                        out=tp[:], in_=srcap,
                        identity=(ident[:] if j < 2 else ident_bf[:]))
                    eng = nc.scalar if j % 2 == 0 else nc.vector
                    if eng is nc.scalar:
                        nc.scalar.copy(out=x2t[:, j, :], in_=tp[:])
                    else:
                        nc.vector.tensor_copy(out=x2t[:, j, :], in_=tp[:])

                # ---- h = relu(x2 @ W2 + b2) ----
                h_ps = accp.tile([P, H], F32, tag="h_ps")
                for j in range(4):
                    nc.tensor.matmul(
                        out=h_ps[:], lhsT=x2t[:, j, :], rhs=w2c(j),
                        start=(j == 0), stop=(j == 3))
                h_sb = work.tile([P, H], F32, tag="h_sb")
                nc.vector.tensor_tensor(
                    out=h_sb[:], in0=h_ps[:], in1=b2_f,
                    op=mybir.AluOpType.add)
                nc.vector.tensor_scalar_max(
                    out=h_sb[:], in0=h_sb[:], scalar1=0.0)

                # ---- layernorm (g/be folded into Wc'/bc' on host) [DVE] ----
                stats = small.tile([P, 6], F32, tag="stats")
                nc.vector.bn_stats(out=stats[:], in_=h_sb[:])
                mv = small.tile([P, 2], F32, tag="mv")
                nc.vector.bn_aggr(out=mv[:], in_=stats[:])
                std = small.tile([P, 1], F32, tag="std")
                nc.scalar.activation(
                    out=std[:], in_=mv[:, 1:2],
                    func=mybir.ActivationFunctionType.Sqrt, bias=eps_t[:])
                rstd = small.tile([P, 1], F32, tag="rstd")
                nc.vector.reciprocal(out=rstd[:], in_=std[:])
                xn = work.tile([P, H], F32, tag="xn")
                nc.vector.tensor_scalar(
                    out=xn[:], in0=h_sb[:],
                    scalar1=mv[:, 0:1], scalar2=rstd[:],
                    op0=mybir.AluOpType.subtract, op1=mybir.AluOpType.mult)

                # ---- logits = xn @ Wc' + bc' ----
                h2t = work.tile([P, 2, P], BF16, tag="h2t")
                for j in range(2):
                    tp2 = tps.tile([P, P], F32, tag="tps", name=f"tp2{j}")
                    nc.tensor.transpose(
                        out=tp2[:], in_=xn[:, j * P:(j + 1) * P],
                        identity=ident[:])
                    if j == 0:
                        nc.scalar.copy(out=h2t[:, j, :], in_=tp2[:])
                    else:
                        nc.vector.tensor_copy(out=h2t[:, j, :], in_=tp2[:])
                o_ps = outp.tile([P, C], F32, tag="o_ps")
                for j in range(2):
                    nc.tensor.matmul(
                        out=o_ps[:], lhsT=h2t[:, j, :], rhs=wcc(j),
                        start=(j == 0), stop=(j == 1))
                ob = work.tile([P, C], F32, tag="ob")
                nc.vector.tensor_tensor(
                    out=ob[:], in0=o_ps[:], in1=bc_f,
                    op=mybir.AluOpType.add)

                # ---- softmax ----
                mx = small.tile([P, 1], F32, tag="mx")
                nc.vector.tensor_reduce(
                    out=mx[:], in_=ob[:],
                    axis=mybir.AxisListType.X, op=mybir.AluOpType.max)
                nmx = small.tile([P, 1], F32, tag="nmx")
                nc.vector.tensor_scalar_mul(out=nmx[:], in0=mx[:], scalar1=-1.0)
                esb = work.tile([P, C], F32, tag="esb")
                ssum = small.tile([P, 1], F32, tag="ssum")
                nc.scalar.activation(
                    out=esb[:], in_=ob[:],
                    func=mybir.ActivationFunctionType.Exp,
                    bias=nmx[:], accum_out=ssum[:])
                rsum = small.tile([P, 1], F32, tag="rsum")
                nc.vector.reciprocal(out=rsum[:], in_=ssum[:])
                res = work.tile([P, C], F32, tag="res")
                nc.vector.tensor_scalar_mul(out=res[:], in0=esb[:], scalar1=rsum[:])
                nc.sync.dma_start(out=out_d[t * P:(t + 1) * P, :], in_=res[:])

    nc.compile()
    return nc


def _get_nc():
    if "nc" not in _CACHE:
        _CACHE["nc"] = _build()
    return _CACHE["nc"]


def _prep_inputs(node_batch, neigh_idx, neigh_mask, feat, hidden1,
                 W2, b2, g2, be2, Wc, bc):
    node_batch = np.asarray(node_batch).astype(np.int32)
    idx2 = np.asarray(neigh_idx[1]).astype(np.int32)        # [B, S]
    m2 = np.asarray(neigh_mask[1]).astype(bool)             # [B, S]
    feat = np.asarray(feat, dtype=np.float32)
    hidden1 = np.asarray(hidden1, dtype=np.float32)
    W2 = np.asarray(W2, dtype=np.float32)
    b2 = np.asarray(b2, dtype=np.float32)
    g2 = np.asarray(g2, dtype=np.float32)
    be2 = np.asarray(be2, dtype=np.float32)
    Wc = np.asarray(Wc, dtype=np.float32)
    bc = np.asarray(bc, dtype=np.float32)

    import ml_dtypes
    bf16 = ml_dtypes.bfloat16
    feat_ext = np.zeros((NPAD, F), bf16)
    feat_ext[:N] = feat.astype(bf16)
    hidden1 = hidden1.astype(bf16)
    idx_eff = np.where(m2, idx2, N).astype(np.int32)        # invalid -> zero row
    wc_p = (g2[:, None] * Wc).astype(np.float32)
    bc_p = (be2 @ Wc + bc).astype(np.float32)
    mask_f = m2.astype(np.float32)

    wpack = np.empty((P, WPCOLS), np.float32)
    wpack[:, WP_W2:WP_WC] = W2.reshape(4, P, H).transpose(1, 0, 2) \
                              .reshape(P, 4 * H)
    wpack[:, WP_WC:WP_B2] = wc_p.reshape(2, P, C).transpose(1, 0, 2) \
                                .reshape(P, 2 * C)
    wpack[:, WP_B2:WP_BC] = np.broadcast_to(b2, (P, H))
    wpack[:, WP_BC:] = np.broadcast_to(bc_p, (P, C))
    wpack_bf = wpack.astype(bf16)
    bpack = np.empty((P, H + C), np.float32)
    bpack[:, 0:H] = np.broadcast_to(b2, (P, H))
    bpack[:, H:] = np.broadcast_to(bc_p, (P, C))

    in_maps = []
    for c in range(NCORES):
        lo = c * BL
        meta = np.empty((P, MCOLS), np.int32)
        meta[:, MC_IDX:MC_NBI] = (
            idx_eff[lo:lo + BL].reshape(NT, P, S).transpose(1, 0, 2)
            .reshape(P, NT * S))
        meta[:, MC_NBI:MC_MSK] = node_batch[lo:lo + BL].reshape(NT, P).T
        meta[:, MC_MSK:] = (
            mask_f[lo:lo + BL].reshape(NT, P, S).transpose(1, 0, 2)
            .reshape(P, NT * S).view(np.int32))
        in_maps.append({
            "feat": feat_ext, "hid": hidden1, "meta": meta,
            "wpack": wpack_bf, "bpack": bpack,
        })
    return in_maps


def kernel(node_batch, neigh_idx, neigh_mask, feat, hidden1,
           W1, b1, g1, be1, W2, b2, g2, be2, Wc, bc, **extra):
    in_maps = _prep_inputs(node_batch, neigh_idx, neigh_mask, feat, hidden1,
                           W2, b2, g2, be2, Wc, bc)
    nc = _get_nc()
    r = run_bass_kernel_spmd(nc, in_maps, core_ids=list(range(NCORES)),
                             **_CACHE.get("run_kwargs", {}))
    out = np.concatenate([r.results[c]["out"] for c in range(NCORES)], axis=0)
    _CACHE["last_result"] = r
    return out
